# revision 1
# baseline (speedup 1.0000x reference)
"""L2-distance self-attention (B=2, N=2048, D=1024, H=16) on 8 trn2 NeuronCores.

Sharding: core c handles batch c//4 and heads 4*(c%4) .. 4*(c%4)+4.
Each core computes its 4 heads end-to-end (q/k/v projection, L2 softmax
attention, per-head output projection partial) and returns a (2048, 1024)
fp32 partial of the output projection; the host sums the 4 partials per
batch and adds bo.

Math per core (hd = 64, heads h = 0..3):
  qb = x @ wq_h.T + bq_h            (computed transposed: (64, N); bias via
                                     ones-row augmentation of the K dim)
  kb2 = -2*(x @ wk_h.T + bk_h)      (host pre-scales wk, bk by -2)
  d2[j,i] = q2[i] + k2[j] - 2*qk    via one K=66 matmul:
     lhsT = k_stat rows [kb2 (64); ones; k2],  rhs = q_aug rows [qb (64); q2; ones]
  s = sqrt(d2)   (ACT, PSUM->SBUF fp16; exp(-s) needs no max-subtract since s>=0)
  e = exp(-s)    (ACT, strided i-chunk read so PV can consume i-chunk major)
  oT_aug = v_aug.T @ e  with v_aug = [v | ones] -> row 64 = softmax denominator
  y_h = oT_h.T @ woT_h ; y_acc += y_h * (1/den[i])   (DVE fused multiply-add)
"""

import sys

for p in ("/opt/trn_rl_repo", "/root/.axon_site/_ro/trn_rl_repo"):
    if p not in sys.path:
        sys.path.append(p)

import numpy as np

B, N, D, H = 2, 2048, 1024, 16
HD = 64          # head dim
HPC = 4          # heads per core
HS = HPC * HD    # head-group width per core (256)
NB = N // 128    # 16 j/i blocks
IC = N // 512    # 4 projection moving chunks
KB = D // 128    # 8 contraction blocks for projections
EC = 256         # exp/PV i-chunk width
NEC = N // EC    # 8 exp chunks per head

_CACHE = {}


def _build(loop_n=None):
    import concourse.bacc as bacc
    import concourse.mybir as mybir
    import concourse.tile as tile

    dt = mybir.dt
    AF = mybir.ActivationFunctionType
    ALU = mybir.AluOpType

    nc = bacc.Bacc("TRN2", target_bir_lowering=False, debug=False)

    # ---- DRAM I/O (per core) ----
    xT = nc.dram_tensor("xT", [D, N], dt.float16, kind="ExternalInput")
    wq = nc.dram_tensor("wq_aug", [D + 1, HS], dt.float16, kind="ExternalInput")
    wk = nc.dram_tensor("wk_aug", [D + 1, HS], dt.float16, kind="ExternalInput")
    wv = nc.dram_tensor("wv_aug", [D + 1, HS], dt.float16, kind="ExternalInput")
    wo = nc.dram_tensor("woT", [HS, D], dt.float16, kind="ExternalInput")
    y = nc.dram_tensor("y", [N, D], dt.float32, kind="ExternalOutput")

    with tile.TileContext(nc) as tc:
        with (
            tc.tile_pool(name="cst", bufs=1) as cst,
            tc.tile_pool(name="u4", bufs=9) as u4,      # 4KB slots: xt -> oT -> y
            tc.tile_pool(name="wp", bufs=1) as wp,
            tc.tile_pool(name="wop", bufs=1) as wop,
            tc.tile_pool(name="aug", bufs=1) as aug,
            tc.tile_pool(name="dp", bufs=1) as dpool,
            tc.tile_pool(name="spool", bufs=1) as spool,
            tc.tile_pool(name="e8", bufs=2) as e8,      # 8KB slots: sqtmp / e
            tc.tile_pool(name="psum", bufs=2, space="PSUM") as ps,
        ):
            # ---- constants ----
            ones_row = cst.tile([1, 512], dt.float16, tag="ones_row")
            nc.gpsimd.memset(ones_row[:], 1.0)
            # E66 matrices: ones-matmul producers for the two augmentation
            # rows (64, 65) of q_aug / k_stat. q_aug: row 64 = q2, row 65 = 1.
            # k_stat: row 64 = 1, row 65 = 0.25*sum(kb2^2) = k2.
            e66q = cst.tile([64, 66], dt.float16, tag="e66q")
            nc.gpsimd.memset(e66q[:], 0.0)
            nc.gpsimd.memset(e66q[:, 64:65], 1.0)
            e66k = cst.tile([64, 66], dt.float16, tag="e66k")
            nc.gpsimd.memset(e66k[:], 0.0)
            nc.gpsimd.memset(e66k[:, 65:66], 0.25)
            e66aq = cst.tile([1, 66], dt.float16, tag="e66aq")  # ones into row 65
            nc.gpsimd.memset(e66aq[:], 0.0)
            nc.gpsimd.memset(e66aq[:, 65:66], 1.0)
            e66ak = cst.tile([1, 66], dt.float16, tag="e66ak")  # ones into row 64
            nc.gpsimd.memset(e66ak[:], 0.0)
            nc.gpsimd.memset(e66ak[:, 64:65], 1.0)

            # PE warmup: dependency-free matmuls keep the tensor engine busy
            # through the input-DMA window so real matmuls start at full clock
            for w in range(2):
                wup = ps.tile([128, 512], dt.float32, tag="big", name="wup")
                for r in range(12):
                    nc.tensor.matmul(
                        wup[:], ones_row[0:1, 0:128], ones_row[0:1, :],
                        start=(r == 0), stop=(r == 11),
                    )

            # ---- load inputs (order tuned so q-proj ic0 can start ASAP;
            # wv/wo go through the gpsimd DGE queue to parallelize issue) ----
            xt = [u4.tile([128, N], dt.float16, tag="u4", name=f"xt{k}") for k in range(KB)]
            wq_all = wp.tile([128, KB * HS], dt.float16, tag="wq_all")
            wk_all = wp.tile([128, KB * HS], dt.float16, tag="wk_all")
            wv_all = wp.tile([128, KB * HS], dt.float16, tag="wv_all")
            brows = wp.tile([1, 3 * HS], dt.float16, tag="brows")
            nc.sync.dma_start(brows[:, 0:HS], wq[D : D + 1, :])
            nc.sync.dma_start(brows[:, HS : 2 * HS], wk[D : D + 1, :])
            for k in range(KB):
                nc.sync.dma_start(
                    wq_all[:, k * HS : (k + 1) * HS], wq[k * 128 : (k + 1) * 128, :]
                )
                nc.sync.dma_start(xt[k][:], xT[k * 128 : (k + 1) * 128, :])
            for k in range(KB):
                nc.sync.dma_start(
                    wk_all[:, k * HS : (k + 1) * HS], wk[k * 128 : (k + 1) * 128, :]
                )
            nc.gpsimd.dma_start(brows[:, 2 * HS : 3 * HS], wv[D : D + 1, :])
            for k in range(KB):
                nc.gpsimd.dma_start(
                    wv_all[:, k * HS : (k + 1) * HS], wv[k * 128 : (k + 1) * 128, :]
                )

            # ---- per-head augmented tiles ----
            q_aug = [aug.tile([66, N], dt.float16, tag=f"qa{h}", name=f"qa{h}") for h in range(HPC)]
            k_stat = [aug.tile([66, N], dt.float16, tag=f"ks{h}", name=f"ks{h}") for h in range(HPC)]
            # v tiles: per jb, 4 head-blocks of [v(64) | ones]
            v_jb = [aug.tile([128, HPC * 65], dt.float16, tag=f"v{jb}", name=f"v{jb}") for jb in range(NB)]
            for jb in range(NB):
                nc.gpsimd.memset(
                    v_jb[jb][:].rearrange("p (b d) -> p b d", d=65)[:, :, 64:65], 1.0
                )

            # ---- q/k projections (Mblk = head pair), bias via extra K row ----
            def proj_qk_ic(w_all, boff, dest, m, ic):
                # heads 2m, 2m+1 ; psum (128 d, 512 i), one i-chunk
                p = ps.tile([128, 512], dt.float32, tag="big", name="pp")
                for k in range(KB + 1):
                    if k < KB:
                        lhsT = w_all[:, k * HS + m * 128 : k * HS + (m + 1) * 128]
                        rhs = xt[k][:, ic * 512 : (ic + 1) * 512]
                    else:
                        lhsT = brows[0:1, boff + m * 128 : boff + (m + 1) * 128]
                        rhs = ones_row[0:1, :]
                    nc.tensor.matmul(p[:], lhsT, rhs, start=(k == 0), stop=(k == KB))
                for half in range(2):
                    nc.vector.tensor_copy(
                        dest[2 * m + half][0:64, ic * 512 : (ic + 1) * 512],
                        p[64 * half : 64 * half + 64, :],
                    )

            # q2 (row 64 of q_aug, row 65 ones) / k2 (row 65 of k_stat, row 64 ones)
            def norms_part(h, which):
                src_tile = q_aug[h] if which == "q" else k_stat[h]
                emat = e66q if which == "q" else e66k
                eadd = e66aq if which == "q" else e66ak
                sq = u4.tile([64, N], dt.float16, tag="u4", name="sq")
                nc.vector.tensor_tensor(
                    out=sq[:], in0=src_tile[0:64, :], in1=src_tile[0:64, :],
                    op=ALU.mult,
                )
                p = ps.tile([66, N], dt.float32, tag="big", name="np")
                for ic in range(IC):
                    nc.tensor.matmul(
                        p[:, ic * 512 : (ic + 1) * 512], emat[:],
                        sq[:, ic * 512 : (ic + 1) * 512], start=True, stop=False,
                    )
                    nc.tensor.matmul(
                        p[:, ic * 512 : (ic + 1) * 512], eadd[:],
                        ones_row[0:1, :], start=False, stop=True,
                    )
                nc.vector.tensor_copy(src_tile[64:66, :], p[64:66, :])

            def vp_one(jb):
                p = ps.tile([128, HS], dt.float32, tag="big", name="vp")
                for k in range(KB + 1):
                    if k < KB:
                        lhsT = xt[k][:, jb * 128 : (jb + 1) * 128]
                        rhs = wv_all[:, k * HS : (k + 1) * HS]
                    else:
                        lhsT = ones_row[0:1, 0:128]
                        rhs = brows[0:1, 2 * HS : 3 * HS]
                    nc.tensor.matmul(p[:], lhsT, rhs, start=(k == 0), stop=(k == KB))
                dst = v_jb[jb][:].rearrange("p (h d) -> p h d", d=65)[:, :, 0:64]
                nc.vector.tensor_copy(dst, p[:].rearrange("p (h d) -> p h d", d=64))

            s = spool.tile([128, NB * N], dt.float16, tag="s")
            sv = s[:].rearrange("p (t i) -> p t i", t=NB)
            oTp = [
                aug.tile([128, N], dt.float16, tag="oTp0", name="oTp0"),
                aug.tile([128, N], dt.float16, tag="oTp1", name="oTp1"),
            ]
            raws = [None] * HPC

            def st_one(h, jb):
                st = ps.tile([128, N], dt.float32, tag="big", name="st")
                for ic in range(IC):
                    nc.tensor.matmul(
                        st[:, ic * 512 : (ic + 1) * 512],
                        k_stat[h][0:66, jb * 128 : (jb + 1) * 128],
                        q_aug[h][0:66, ic * 512 : (ic + 1) * 512],
                        start=True, stop=True,
                    )
                nc.scalar.activation(s[:, jb * N : (jb + 1) * N], st[:], AF.Sqrt)

            def exp_chunk(h, c, pv):
                e = e8.tile([128, NB * EC], dt.float16, tag="e8", name="e")
                nc.scalar.activation(
                    e[:].rearrange("p (t i) -> p t i", t=NB),
                    sv[:, :, c * EC : (c + 1) * EC],
                    AF.Exp, scale=-1.0,
                )
                for t in range(NB):
                    nc.tensor.matmul(
                        pv[:, c * EC : (c + 1) * EC],
                        v_jb[t][:, h * 65 : h * 65 + 65],
                        e[:, t * EC : (t + 1) * EC],
                        start=(t == 0), stop=(t == NB - 1),
                    )

            def raw_copy(h, pv):
                raws[h] = u4.tile([65, N], dt.float16, tag="u4", name=f"raw{h}")
                with nc.allow_low_precision(reason="fp16 softmax weights"):
                    nc.vector.tensor_copy(raws[h][:], pv[:])
                    nc.vector.reciprocal(out=dpool_row[:], in_=raws[h][64:65, :])

            def norm_bc_mm(h):
                bc = ps.tile([64, N], dt.float32, tag="big", name="bc")
                for ic in range(IC):
                    nc.tensor.matmul(
                        bc[:, ic * 512 : (ic + 1) * 512],
                        ones_row[0:1, 0:64],
                        dpool_row[0:1, ic * 512 : (ic + 1) * 512],
                        start=True, stop=True,
                    )
                return bc

            def norm_finish(h, bc):
                bcs = u4.tile([64, N], dt.float16, tag="u4", name=f"bcs{h}")
                nc.vector.tensor_copy(bcs[:], bc[:])
                half = 64 * (h % 2)
                nc.vector.tensor_tensor(
                    out=oTp[h // 2][half : half + 64, :],
                    in0=raws[h][0:64, :], in1=bcs[:], op=ALU.mult,
                )

            dpool_row = dpool.tile([1, N], dt.float16, tag="dinrow")

            # ================= emission schedule =================
            # lead-in: head-pair-0 q/k projections + head-0 norms
            for ic in range(IC):
                proj_qk_ic(wq_all, 0, q_aug, 0, ic)
            for ic in range(IC):
                proj_qk_ic(wk_all, HS, k_stat, 0, ic)
            norms_part(0, "q")
            norms_part(0, "k")

            # head-0 S.T/sqrt; v-proj + head-1 norms fill PE gaps
            st_one(0, 0)
            norms_part(1, "q")
            st_one(0, 1)
            norms_part(1, "k")
            for jb in range(2, NB):
                st_one(0, jb)
                vp_one(jb - 2)
            vp_one(NB - 2)
            vp_one(NB - 1)

            # wo loads (no psum; DMA only)
            wotp = [wop.tile([128, D], dt.float16, tag=f"wop{p}", name=f"wop{p}") for p in range(2)]
            for p in range(2):
                nc.gpsimd.dma_start(wotp[p][:], wo[p * 128 : (p + 1) * 128, :])

            # head-0 exp/PV; head-pair-1 projections fill PE gaps
            m1 = [(wq_all, 0, q_aug), (wk_all, HS, k_stat)]
            pv = ps.tile([65, N], dt.float32, tag="big", name="pv")
            for c in range(NEC):
                exp_chunk(0, c, pv)
                w_all_, boff_, dest_ = m1[c // IC]
                proj_qk_ic(w_all_, boff_, dest_, 1, c % IC)
            raw_copy(0, pv)

            # head-1 S.T/sqrt; normalize-0 + head-2/3 norms fill gaps
            st_one(1, 0)
            st_one(1, 1)
            st_one(1, 2)
            bc0 = norm_bc_mm(0)
            st_one(1, 3)
            norm_finish(0, bc0)
            for jb in range(4, NB):
                st_one(1, jb)
                if jb == 4:
                    norms_part(2, "q")
                elif jb == 6:
                    norms_part(2, "k")
                elif jb == 8:
                    norms_part(3, "q")
                elif jb == 10:
                    norms_part(3, "k")

            for h in range(1, HPC - 1):
                pv = ps.tile([65, N], dt.float32, tag="big", name="pv")
                for c in range(NEC):
                    exp_chunk(h, c, pv)
                raw_copy(h, pv)
                st_one(h + 1, 0)
                st_one(h + 1, 1)
                st_one(h + 1, 2)
                bc = norm_bc_mm(h)
                st_one(h + 1, 3)
                norm_finish(h, bc)
                for jb in range(4, NB):
                    st_one(h + 1, jb)

            # last head: normalize chunk-wise right behind each PV chunk so
            # oT is complete almost as soon as the last exp finishes
            pv = ps.tile([65, N], dt.float32, tag="big", name="pv")
            bc = ps.tile([64, N], dt.float32, tag="big", name="bc3")
            h = HPC - 1
            for c in range(NEC):
                exp_chunk(h, c, pv)
                lo, hi = c * EC, (c + 1) * EC
                with nc.allow_low_precision(reason="fp16 softmax weights"):
                    nc.vector.reciprocal(
                        out=dpool_row[0:1, lo:hi], in_=pv[64:65, lo:hi]
                    )
                nc.tensor.matmul(
                    bc[:, lo:hi], ones_row[0:1, 0:64], dpool_row[0:1, lo:hi],
                    start=True, stop=True,
                )
                bcs = u4.tile([64, EC], dt.float16, tag="u4", name="bcs3c")
                nc.vector.tensor_copy(bcs[:], bc[:, lo:hi])
                with nc.allow_low_precision(reason="fp16 softmax weights"):
                    nc.vector.tensor_tensor(
                        out=oTp[h // 2][64 : 128, lo:hi],
                        in0=pv[0:64, lo:hi], in1=bcs[:], op=ALU.mult,
                    )

            # ---- output projection: all heads accumulate in PSUM ----
            def yp_mms(yp, ib, pairs, stop_p):
                for pr in pairs:
                    for fc in range(2):
                        nc.tensor.matmul(
                            yp[:, fc * 512 : (fc + 1) * 512],
                            oTp[pr][:, ib * 128 : (ib + 1) * 128],
                            wotp[pr][:, fc * 512 : (fc + 1) * 512],
                            start=(pr == 0), stop=(pr == stop_p),
                        )

            def yac_out(yp, ib):
                yac = u4.tile([128, D], dt.float32, tag="u4", name="yac")
                if ib % 2 == 0:
                    nc.scalar.copy(yac[:], yp[:])
                else:
                    nc.vector.tensor_copy(yac[:], yp[:])
                nc.sync.dma_start(y[ib * 128 : (ib + 1) * 128, :], yac[:])

            yp0 = ps.tile([128, D], dt.float32, tag="big", name="yp")
            yp_mms(yp0, 0, [0], 1)
            yp1 = ps.tile([128, D], dt.float32, tag="big", name="yp")
            yp_mms(yp1, 1, [0], 1)
            yp_mms(yp0, 0, [1], 1)
            yac_out(yp0, 0)
            yp_mms(yp1, 1, [1], 1)
            yac_out(yp1, 1)
            for ib in range(2, NB):
                yp = ps.tile([128, D], dt.float32, tag="big", name="yp")
                yp_mms(yp, ib, [0, 1], 1)
                yac_out(yp, ib)

    nc.compile()
    return nc


def _prep_in_maps(x, wq, bq, wk, bk, wv, bv, wo):
    f16 = np.float16
    in_maps = []
    xTs = [np.ascontiguousarray(x[b].T).astype(f16) for b in range(B)]
    for c in range(8):
        b, hg = divmod(c, HPC)
        hs = hg * HS
        wq_aug = np.concatenate(
            [wq[hs : hs + HS, :].T, bq[None, hs : hs + HS]], axis=0
        ).astype(f16)
        wk_aug = np.concatenate(
            [-2.0 * wk[hs : hs + HS, :].T, -2.0 * bk[None, hs : hs + HS]], axis=0
        ).astype(f16)
        wv_aug = np.concatenate(
            [wv[hs : hs + HS, :].T, bv[None, hs : hs + HS]], axis=0
        ).astype(f16)
        woT = np.ascontiguousarray(wo[:, hs : hs + HS].T).astype(f16)
        in_maps.append(
            {
                "xT": xTs[b],
                "wq_aug": np.ascontiguousarray(wq_aug),
                "wk_aug": np.ascontiguousarray(wk_aug),
                "wv_aug": np.ascontiguousarray(wv_aug),
                "woT": woT,
            }
        )
    return in_maps


def _get_nc():
    if "nc" not in _CACHE:
        _CACHE["nc"] = _build()
    return _CACHE["nc"]


def run(inputs, trace=False, **trace_kwargs):
    """Run on 8 cores; returns (full_output, BassKernelResults)."""
    from concourse.bass_utils import run_bass_kernel_spmd

    nc = _get_nc()
    in_maps = _prep_in_maps(
        np.asarray(inputs["x"], np.float32),
        np.asarray(inputs["wq"], np.float32), np.asarray(inputs["bq"], np.float32),
        np.asarray(inputs["wk"], np.float32), np.asarray(inputs["bk"], np.float32),
        np.asarray(inputs["wv"], np.float32), np.asarray(inputs["bv"], np.float32),
        np.asarray(inputs["wo"], np.float32),
    )
    res = run_bass_kernel_spmd(nc, in_maps, list(range(8)), trace=trace, **trace_kwargs)
    bo = np.asarray(inputs["bo"], np.float32)
    out = np.empty((B, N, D), np.float32)
    for b in range(B):
        acc = res.results[b * HPC]["y"].astype(np.float32)
        for c in range(b * HPC + 1, (b + 1) * HPC):
            acc = acc + res.results[c]["y"]
        out[b] = acc + bo
    return out, res


def kernel(**inputs) -> np.ndarray:
    out, _ = run(inputs, trace=False)
    return out


if __name__ == "__main__":
    rng = np.random.default_rng(0)
    ins = {
        "x": rng.standard_normal((B, N, D)).astype(np.float32),
        "wq": (rng.standard_normal((D, D)) * 0.02).astype(np.float32),
        "bq": (rng.standard_normal(D) * 0.02).astype(np.float32),
        "wk": (rng.standard_normal((D, D)) * 0.02).astype(np.float32),
        "bk": (rng.standard_normal(D) * 0.02).astype(np.float32),
        "wv": (rng.standard_normal((D, D)) * 0.02).astype(np.float32),
        "bv": (rng.standard_normal(D) * 0.02).astype(np.float32),
        "wo": (rng.standard_normal((D, D)) * 0.02).astype(np.float32),
        "bo": (rng.standard_normal(D) * 0.02).astype(np.float32),
    }
    print(kernel(**ins).shape)



# revision 9
# speedup vs baseline: 1.1820x; 1.1820x over previous
"""L2-distance self-attention (B=2, N=2048, D=1024, H=16) on 8 trn2 NeuronCores.

Sharding: core c handles batch c//4 and heads 4*(c%4) .. 4*(c%4)+4.
Each core computes its 4 heads end-to-end and writes a (2048, 1024) fp16
partial of the output projection; the host sums the 4 partials per batch
and adds bo.

v2 design notes (ACT-engine-bound restructure):
  - ACT (scalar) engine is the bottleneck: sqrt + exp over N^2 per head is
    ~60us/head at 1 elem/cycle/lane.  Everything else drafts behind it.
  - d2 computed via one K=66 matmul per (head, jb):
      lhsT = k_stat rows [kb2 (64); ones; k2],  rhs = q_aug rows [qb; q2; ones]
  - DVE casts d2 PSUM->SBUF fp16 (dodges ACT's PSUM-read penalty), ACT does
    sqrt on fp16 pairs (FD=4096), then exp on pairs.  2 table switches/head.
  - PSUM is managed as two explicit 4-bank tags (ps0/ps1) that alternate.
  - Last head does i-chunked exp/PV/normalize/y-proj so the output projection
    streams out behind the final exps (no serial tail).
"""

import sys

for p in ("/opt/trn_rl_repo", "/root/.axon_site/_ro/trn_rl_repo"):
    if p not in sys.path:
        sys.path.append(p)

import numpy as np

B, N, D, H = 2, 2048, 1024, 16
HD = 64          # head dim
HPC = 4          # heads per core
HS = HPC * HD    # head-group width per core (256)
NB = N // 128    # 16 j-blocks
IC = N // 512    # 4 i-chunks of 512
KB = D // 128    # 8 contraction blocks for projections
EC = 256         # head-3 exp/PV i-chunk width
NEC = N // EC    # 8 chunks

_CACHE = {}


def _build():
    import concourse.bacc as bacc
    import concourse.mybir as mybir
    import concourse.tile as tile

    dt = mybir.dt
    AF = mybir.ActivationFunctionType
    ALU = mybir.AluOpType

    nc = bacc.Bacc("TRN2", target_bir_lowering=False, debug=False)

    xT = nc.dram_tensor("xT", [D, N], dt.float16, kind="ExternalInput")
    wq = nc.dram_tensor("wq_aug", [D + 1, HS], dt.float16, kind="ExternalInput")
    wk = nc.dram_tensor("wk_aug", [D + 1, HS], dt.float16, kind="ExternalInput")
    wv = nc.dram_tensor("wv_aug", [D + 1, HS], dt.float16, kind="ExternalInput")
    wo = nc.dram_tensor("woT", [HS, D], dt.float16, kind="ExternalInput")
    y = nc.dram_tensor("y", [N, D], dt.float16, kind="ExternalOutput")

    with tile.TileContext(nc) as tc:
        with (
            tc.tile_pool(name="cst", bufs=1) as cst,
            tc.tile_pool(name="wp", bufs=1) as wp,
            tc.tile_pool(name="xp", bufs=1) as xp,
            tc.tile_pool(name="aug", bufs=1) as aug,
            tc.tile_pool(name="sqp", bufs=1) as sqp,
            tc.tile_pool(name="spool", bufs=1) as spool,
            tc.tile_pool(name="sc", bufs=2) as sc,
            tc.tile_pool(name="sml", bufs=1) as sml,
            tc.tile_pool(name="yo", bufs=2) as yo,
            tc.tile_pool(name="ps", bufs=1, space="PSUM") as ps,
        ):
            # ---- explicit 2-tag PSUM ring ([128,2048] fp32 = 4 banks each) ----
            # pin= forces a parity (for fillers that must not collide with a
            # long-lived pv tile); default alternates.
            par = [0]

            def ps_tile(pdim, fdim, name, pin=None):
                pr_ = par[0] if pin is None else pin
                t = ps.tile([pdim, fdim], dt.float32, tag=f"ps{pr_}", name=name)
                if pin is None:
                    par[0] ^= 1
                return t

            # ---- constants ----
            ones_row = cst.tile([1, 512], dt.float16, tag="ones_row")
            nc.gpsimd.memset(ones_row[:], 1.0)
            dum = cst.tile([1, 16], dt.float16, tag="dum")
            dumo = cst.tile([1, 16], dt.float16, tag="dumo")
            nc.gpsimd.memset(dum[:], 1.0)
            # e65 matrices: single-matmul producers for q2/k2 + the const row.
            # np = e65.T @ sq_aug with sq_aug rows [src^2 (64); ones].
            e65q = cst.tile([65, 66], dt.float16, tag="e65q")
            nc.gpsimd.memset(e65q[:], 0.0)
            nc.gpsimd.memset(e65q[0:64, 64:65], 1.0)   # row64 = q2
            nc.gpsimd.memset(e65q[64:65, 65:66], 1.0)  # row65 = ones
            e65k = cst.tile([65, 66], dt.float16, tag="e65k")
            nc.gpsimd.memset(e65k[:], 0.0)
            nc.gpsimd.memset(e65k[0:64, 65:66], 0.25)  # row65 = k2 (kb2 = -2k)
            nc.gpsimd.memset(e65k[64:65, 64:65], 1.0)  # row64 = ones

            # ACT table warm: exp first, sqrt second -> sqrt resident when the
            # first real sqrt batch issues.
            nc.scalar.activation(dumo[:], dum[:], AF.Exp)
            nc.scalar.activation(dumo[:], dum[:], AF.Sqrt)

            # PE warmup: dependency-free matmuls through the DMA window
            for w in range(2):
                wup = ps_tile(128, 512, "wup")
                for r in range(12):
                    nc.tensor.matmul(
                        wup[:], ones_row[0:1, 0:128], ones_row[0:1, :],
                        start=(r == 0), stop=(r == 11),
                    )

            # ---- input DMAs ----
            xt = [xp.tile([128, N], dt.float16, tag=f"xt{k}", name=f"xt{k}") for k in range(KB)]
            wq_all = wp.tile([128, KB * HS], dt.float16, tag="wq_all")
            wk_all = wp.tile([128, KB * HS], dt.float16, tag="wk_all")
            wv_all = wp.tile([128, KB * HS], dt.float16, tag="wv_all")
            brows = wp.tile([1, 3 * HS], dt.float16, tag="brows")
            nc.sync.dma_start(brows[:, 0:HS], wq[D : D + 1, :])
            nc.sync.dma_start(brows[:, HS : 2 * HS], wk[D : D + 1, :])
            for k in range(KB):
                nc.sync.dma_start(
                    wq_all[:, k * HS : (k + 1) * HS], wq[k * 128 : (k + 1) * 128, :]
                )
                nc.sync.dma_start(xt[k][:], xT[k * 128 : (k + 1) * 128, :])
            for k in range(KB):
                nc.sync.dma_start(
                    wk_all[:, k * HS : (k + 1) * HS], wk[k * 128 : (k + 1) * 128, :]
                )
            nc.gpsimd.dma_start(brows[:, 2 * HS : 3 * HS], wv[D : D + 1, :])
            for k in range(KB):
                nc.gpsimd.dma_start(
                    wv_all[:, k * HS : (k + 1) * HS], wv[k * 128 : (k + 1) * 128, :]
                )
            wotp = [wp.tile([128, D], dt.float16, tag=f"wop{p}", name=f"wop{p}") for p in range(2)]
            for p in range(2):
                nc.gpsimd.dma_start(wotp[p][:], wo[p * 128 : (p + 1) * 128, :])

            # ---- persistent tiles ----
            q_aug = [aug.tile([66, N], dt.float16, tag=f"qa{h}", name=f"qa{h}") for h in range(HPC)]
            k_stat = [aug.tile([66, N], dt.float16, tag=f"ks{h}", name=f"ks{h}") for h in range(HPC)]
            v_jb = [aug.tile([128, HPC * 65], dt.float16, tag=f"v{jb}", name=f"v{jb}") for jb in range(NB)]
            for jb in range(NB):
                nc.gpsimd.memset(
                    v_jb[jb][:].rearrange("p (b d) -> p b d", d=65)[:, :, 64:65], 1.0
                )
            oTp = [
                aug.tile([128, N], dt.float16, tag="oTp0", name="oTp0"),
                aug.tile([128, N], dt.float16, tag="oTp1", name="oTp1"),
            ]
            sqt = sqp.tile([65, N], dt.float16, tag="sqt")
            nc.gpsimd.memset(sqt[64:65, :], 1.0)
            s = spool.tile([128, NB * N], dt.float16, tag="s")
            sv = s[:].rearrange("p (t i) -> p t i", t=NB)
            raws = sml.tile([65, N], dt.float16, tag="raws")
            dinv = sml.tile([1, N], dt.float16, tag="dinv")
            bcs = sml.tile([64, N], dt.float16, tag="bcs")

            # ---- building blocks ----
            def proj_m_big(w_all, boff, dest, m):
                # heads 2m, 2m+1; k-outer so DMA arrival pipelines
                p = ps_tile(128, N, "projp")
                for k in range(KB):
                    for ic in range(IC):
                        nc.tensor.matmul(
                            p[:, ic * 512 : (ic + 1) * 512],
                            w_all[:, k * HS + m * 128 : k * HS + (m + 1) * 128],
                            xt[k][:, ic * 512 : (ic + 1) * 512],
                            start=(k == 0), stop=False,
                        )
                for ic in range(IC):
                    nc.tensor.matmul(
                        p[:, ic * 512 : (ic + 1) * 512],
                        brows[0:1, boff + m * 128 : boff + (m + 1) * 128],
                        ones_row[0:1, :],
                        start=False, stop=True,
                    )
                for half in range(2):
                    nc.vector.tensor_copy(
                        dest[2 * m + half][0:64, :],
                        p[64 * half : 64 * half + 64, :],
                    )

            def proj_m_ic(w_all, boff, dest, m, ic, pin=None):
                # per-ic variant (short PSUM hold) for filler use
                p = ps_tile(128, 512, "projic", pin=pin)
                for k in range(KB + 1):
                    if k < KB:
                        lhsT = w_all[:, k * HS + m * 128 : k * HS + (m + 1) * 128]
                        rhs = xt[k][:, ic * 512 : (ic + 1) * 512]
                    else:
                        lhsT = brows[0:1, boff + m * 128 : boff + (m + 1) * 128]
                        rhs = ones_row[0:1, :]
                    nc.tensor.matmul(p[:], lhsT, rhs, start=(k == 0), stop=(k == KB))
                for half in range(2):
                    nc.vector.tensor_copy(
                        dest[2 * m + half][0:64, ic * 512 : (ic + 1) * 512],
                        p[64 * half : 64 * half + 64, :],
                    )

            def norms(h, which):
                src = q_aug[h] if which == "q" else k_stat[h]
                emat = e65q if which == "q" else e65k
                nc.vector.tensor_tensor(
                    out=sqt[0:64, :], in0=src[0:64, :], in1=src[0:64, :],
                    op=ALU.mult,
                )
                np_ = ps_tile(66, N, "np")
                for ic in range(IC):
                    nc.tensor.matmul(
                        np_[:, ic * 512 : (ic + 1) * 512], emat[:],
                        sqt[:, ic * 512 : (ic + 1) * 512], start=True, stop=True,
                    )
                nc.vector.tensor_copy(src[64:66, :], np_[64:66, :])

            def vp_one(jb):
                p = ps_tile(128, HS, "vp")
                for k in range(KB + 1):
                    if k < KB:
                        lhsT = xt[k][:, jb * 128 : (jb + 1) * 128]
                        rhs = wv_all[:, k * HS : (k + 1) * HS]
                    else:
                        lhsT = ones_row[0:1, 0:128]
                        rhs = brows[0:1, 2 * HS : 3 * HS]
                    nc.tensor.matmul(p[:], lhsT, rhs, start=(k == 0), stop=(k == KB))
                dst = v_jb[jb][:].rearrange("p (h d) -> p h d", d=65)[:, :, 0:64]
                nc.vector.tensor_copy(dst, p[:].rearrange("p (h d) -> p h d", d=64))

            def st_mms(h, jb):
                st = ps_tile(128, N, "st")
                for ic in range(IC):
                    nc.tensor.matmul(
                        st[:, ic * 512 : (ic + 1) * 512],
                        k_stat[h][0:66, jb * 128 : (jb + 1) * 128],
                        q_aug[h][0:66, ic * 512 : (ic + 1) * 512],
                        start=True, stop=True,
                    )
                return st

            d16 = [None]

            def st_mms_pin(h, jb, pin):
                st = ps.tile([128, N], dt.float32, tag=f"ps{pin}", name="st")
                for ic in range(IC):
                    nc.tensor.matmul(
                        st[:, ic * 512 : (ic + 1) * 512],
                        k_stat[h][0:66, jb * 128 : (jb + 1) * 128],
                        q_aug[h][0:66, ic * 512 : (ic + 1) * 512],
                        start=True, stop=True,
                    )
                return st

            def st_and_cast(h, jb, pin=None):
                st = st_mms(h, jb) if pin is None else st_mms_pin(h, jb, pin)
                if jb % 2 == 0:
                    d16[0] = sc.tile([128, 2 * N], dt.float16, tag="sc8", name="d16")
                nc.vector.tensor_copy(
                    d16[0][:, (jb % 2) * N : (jb % 2 + 1) * N], st[:]
                )
                if jb % 2 == 1:
                    nc.scalar.activation(
                        s[:, (jb - 1) * N : (jb + 1) * N], d16[0][:], AF.Sqrt
                    )

            def exp_pair(h, pr):
                e = sc.tile([128, 2 * N], dt.float16, tag="sc8", name="e")
                nc.scalar.activation(
                    e[:], s[:, pr * 2 * N : (pr + 1) * 2 * N], AF.Exp, scale=-1.0
                )
                return e

            def pv_pair(h, pr, e, pv):
                for tt in range(2):
                    t = 2 * pr + tt
                    for ic in range(IC):
                        nc.tensor.matmul(
                            pv[:, ic * 512 : (ic + 1) * 512],
                            v_jb[t][:, h * 65 : h * 65 + 65],
                            e[:, tt * N + ic * 512 : tt * N + (ic + 1) * 512],
                            start=(t == 0), stop=(t == NB - 1),
                        )

            def raw_copy(h, pv):
                with nc.allow_low_precision(reason="fp16 softmax weights"):
                    nc.vector.tensor_copy(raws[:], pv[:])

            def recip_half(half):
                lo, hi = half * (N // 2), (half + 1) * (N // 2)
                with nc.allow_low_precision(reason="fp16 softmax weights"):
                    nc.vector.reciprocal(out=dinv[0:1, lo:hi], in_=raws[64:65, lo:hi])

            def bc_mm(pin=None):
                bc = ps_tile(64, N, "bc", pin=pin)
                for ic in range(IC):
                    nc.tensor.matmul(
                        bc[:, ic * 512 : (ic + 1) * 512],
                        ones_row[0:1, 0:64],
                        dinv[0:1, ic * 512 : (ic + 1) * 512],
                        start=True, stop=True,
                    )
                return bc

            def norm_fin(h, bc):
                nc.vector.tensor_copy(bcs[:], bc[:])
                half = 64 * (h % 2)
                nc.vector.tensor_tensor(
                    out=oTp[h // 2][half : half + 64, :],
                    in0=raws[0:64, :], in1=bcs[:], op=ALU.mult,
                )

            # ================= emission schedule =================
            # lead-in: heads 0/1 q,k projections + their norms (k-outer,
            # pipelines with input DMA)
            proj_m_big(wq_all, 0, q_aug, 0)
            proj_m_big(wk_all, HS, k_stat, 0)
            norms(0, "q")
            norms(0, "k")
            norms(1, "q")
            norms(1, "k")

            # --- head 0 sqrt phase: ST+cast+sqrt, fillers = v projections ---
            for jb in range(NB):
                st_and_cast(0, jb)
                vp_one(jb)

            # --- head 0 exp phase: exp+PV, fillers = head-pair-1 projections ---
            pvpar = par[0]
            pv = ps_tile(65, N, "pv")
            fil = 1 - pvpar
            for pr in range(NB // 2):
                e = exp_pair(0, pr)
                pv_pair(0, pr, e, pv)
                if pr < 4:
                    proj_m_ic(wq_all, 0, q_aug, 1, pr % IC, pin=fil)
                else:
                    proj_m_ic(wk_all, HS, k_stat, 1, pr % IC, pin=fil)
            st_and_cast(1, 0, pin=fil)
            raw_copy(0, pv)
            st_and_cast(1, 1, pin=pvpar)
            par[0] = fil

            # --- heads 1..2 ---
            for h in range(1, HPC - 1):
                # sqrt phase h: remaining ST tiles; fillers: norms (h==1)
                for jb in range(2, NB):
                    st_and_cast(h, jb)
                    if h == 1 and jb in (3, 5):
                        norms(2, "q" if jb == 3 else "k")
                    if h == 1 and jb in (7, 9):
                        norms(3, "q" if jb == 7 else "k")
                # exp phase h + boundary(h-1) norm work (DVE idle here)
                pvpar = par[0]
                pv = ps_tile(65, N, "pv")
                fil = 1 - pvpar
                for pr in range(NB // 2):
                    e = exp_pair(h, pr)
                    pv_pair(h, pr, e, pv)
                    if pr == 1:
                        recip_half(0)
                    elif pr == 3:
                        recip_half(1)
                    elif pr == 4:
                        bc = bc_mm(pin=fil)
                        norm_fin(h - 1, bc)
                st_and_cast(h + 1, 0, pin=fil)
                raw_copy(h, pv)
                st_and_cast(h + 1, 1, pin=pvpar)
                par[0] = fil

            # --- head 3 sqrt phase + boundary(2) (must finish before y-proj) ---
            for jb in range(2, NB):
                st_and_cast(3, jb)
            recip_half(0)
            recip_half(1)
            bc = bc_mm()
            norm_fin(2, bc)

            # --- head 3: i-chunked exp/PV/normalize/y-proj streams out ---
            pvpar = par[0]
            pv3 = ps_tile(65, N, "pv3")
            fil = 1 - pvpar
            h = HPC - 1
            for c in range(NEC):
                ec = sc.tile([128, NB * EC], dt.float16, tag="sc8", name="ec")
                nc.scalar.activation(
                    ec[:].rearrange("p (t i) -> p t i", t=NB),
                    sv[:, :, c * EC : (c + 1) * EC],
                    AF.Exp, scale=-1.0,
                )
                for t in range(NB):
                    nc.tensor.matmul(
                        pv3[:, c * EC : (c + 1) * EC],
                        v_jb[t][:, h * 65 : h * 65 + 65],
                        ec[:, t * EC : (t + 1) * EC],
                        start=(t == 0), stop=(t == NB - 1),
                    )
                lo, hi = c * EC, (c + 1) * EC
                with nc.allow_low_precision(reason="fp16 softmax weights"):
                    nc.vector.reciprocal(out=dinv[0:1, lo:hi], in_=pv3[64:65, lo:hi])
                bc3 = ps_tile(64, EC, "bc3", pin=fil)
                nc.tensor.matmul(
                    bc3[:], ones_row[0:1, 0:64], dinv[0:1, lo:hi],
                    start=True, stop=True,
                )
                nc.vector.tensor_copy(bcs[:, lo:hi], bc3[:])
                with nc.allow_low_precision(reason="fp16 softmax weights"):
                    nc.vector.tensor_tensor(
                        out=oTp[1][64:128, lo:hi],
                        in0=pv3[0:64, lo:hi], in1=bcs[:, lo:hi], op=ALU.mult,
                    )
                for ib in (2 * c, 2 * c + 1):
                    yp = ps_tile(128, D, "yp", pin=fil)
                    for prj in range(2):
                        for fc in range(2):
                            nc.tensor.matmul(
                                yp[:, fc * 512 : (fc + 1) * 512],
                                oTp[prj][:, ib * 128 : (ib + 1) * 128],
                                wotp[prj][:, fc * 512 : (fc + 1) * 512],
                                start=(prj == 0), stop=(prj == 1),
                            )
                    yac = yo.tile([128, D], dt.float16, tag="yac", name="yac")
                    with nc.allow_low_precision(reason="fp16 output partials"):
                        nc.vector.tensor_copy(yac[:], yp[:])
                    nc.sync.dma_start(y[ib * 128 : (ib + 1) * 128, :], yac[:])

    nc.compile()
    return nc


def _prep_in_maps(x, wq, bq, wk, bk, wv, bv, wo):
    f16 = np.float16
    in_maps = []
    xTs = [np.ascontiguousarray(x[b].T).astype(f16) for b in range(B)]
    for c in range(8):
        b, hg = divmod(c, HPC)
        hs = hg * HS
        wq_aug = np.concatenate(
            [wq[hs : hs + HS, :].T, bq[None, hs : hs + HS]], axis=0
        ).astype(f16)
        wk_aug = np.concatenate(
            [-2.0 * wk[hs : hs + HS, :].T, -2.0 * bk[None, hs : hs + HS]], axis=0
        ).astype(f16)
        wv_aug = np.concatenate(
            [wv[hs : hs + HS, :].T, bv[None, hs : hs + HS]], axis=0
        ).astype(f16)
        woT = np.ascontiguousarray(wo[:, hs : hs + HS].T).astype(f16)
        in_maps.append(
            {
                "xT": xTs[b],
                "wq_aug": np.ascontiguousarray(wq_aug),
                "wk_aug": np.ascontiguousarray(wk_aug),
                "wv_aug": np.ascontiguousarray(wv_aug),
                "woT": woT,
            }
        )
    return in_maps


def _get_nc():
    if "nc" not in _CACHE:
        _CACHE["nc"] = _build()
    return _CACHE["nc"]


def run(inputs, trace=False, **trace_kwargs):
    """Run on 8 cores; returns (full_output, BassKernelResults)."""
    from concourse.bass_utils import run_bass_kernel_spmd

    nc = _get_nc()
    in_maps = _prep_in_maps(
        np.asarray(inputs["x"], np.float32),
        np.asarray(inputs["wq"], np.float32), np.asarray(inputs["bq"], np.float32),
        np.asarray(inputs["wk"], np.float32), np.asarray(inputs["bk"], np.float32),
        np.asarray(inputs["wv"], np.float32), np.asarray(inputs["bv"], np.float32),
        np.asarray(inputs["wo"], np.float32),
    )
    res = run_bass_kernel_spmd(nc, in_maps, list(range(8)), trace=trace, **trace_kwargs)
    bo = np.asarray(inputs["bo"], np.float32)
    out = np.empty((B, N, D), np.float32)
    for b in range(B):
        acc = res.results[b * HPC]["y"].astype(np.float32)
        for c in range(b * HPC + 1, (b + 1) * HPC):
            acc = acc + res.results[c]["y"].astype(np.float32)
        out[b] = acc + bo
    return out, res


def kernel(**inputs) -> np.ndarray:
    out, _ = run(inputs, trace=False)
    return out


if __name__ == "__main__":
    rng = np.random.default_rng(0)
    ins = {
        "x": rng.standard_normal((B, N, D)).astype(np.float32),
        "wq": (rng.standard_normal((D, D)) * 0.02).astype(np.float32),
        "bq": (rng.standard_normal(D) * 0.02).astype(np.float32),
        "wk": (rng.standard_normal((D, D)) * 0.02).astype(np.float32),
        "bk": (rng.standard_normal(D) * 0.02).astype(np.float32),
        "wv": (rng.standard_normal((D, D)) * 0.02).astype(np.float32),
        "bv": (rng.standard_normal(D) * 0.02).astype(np.float32),
        "wo": (rng.standard_normal((D, D)) * 0.02).astype(np.float32),
        "bo": (rng.standard_normal(D) * 0.02).astype(np.float32),
    }
    print(kernel(**ins).shape)
